# revision 1
# baseline (speedup 1.0000x reference)
"""Trainium2 Bass kernel for AttentionPooling.

Math (per batch element b):
  xf = x[b] reshaped [C, N] with C=512, N=4096
  q = wq@xf + bq ; k = wk@xf + bk ; v = wv@xf + bv          (each [64, N])
  logits = q @ k^T  [64, 64];  attn = softmax(logits, axis over rows o)
  out[b] = mean_n(attn @ v) = attn @ mean_n(v)              ([64])

Because attn does not depend on n, mean_n(attn @ v) = attn @ vbar with
vbar = mean_n(v) — the heavy [64, N] attn@v product collapses to a [64]
vector, so only the q/k projections and a 64x64 logits product are real
work.  Implementation, per batch element:

  - x, wq, wk, wv are shipped as fp16 (10-bit mantissa, same class as
    tf32/f32r device rounding; empirically 4.5e-3 end-to-end rel err)
    which halves the HBM traffic for x — the dominant memory cost.
  - X-STATIONARY fused projection: each [128, 128] x chunk-subtile is
    the matmul weight; ONE matmul per (chunk, subtile) streams the fused
    [wqT | wvT | wkT] 192-column block through it, producing
    [q0T | v0T | k0T] directly in n-major layout (no transposes, half
    the PE instructions), fp32 PSUM accumulation over the 4 C-chunks.
  - One 3D-AP vector-engine copy per projection tile moves all 4
    subtiles' [ones | qT | vT | kT | ones] attention operands to SBUF;
    the ones-augmented [65, 129] attention matmuls (lhsT=[kT|1],
    rhs=[1|qT|vT], accumulated over all 32 n-subtiles) produce L0^T AND
    sum_n k0 (column 0), sum_n q0, sum_n v0 (partition-64 row) — the
    bias-correction and pooling sums ride the same accumulation.  They
    run one projection tile behind so they never stall on the copies.
  - Bias corrections applied analytically on the 64x64 logits:
      L^T = L0^T + bq (x) (sk + N bk) + bk (x) sq
    (valid because logits(q0+bq, k0+bk) is bilinear and attn/softmax only
    needs the full L^T).  sq is broadcast and the v-sum row transposed
    off PSUM partition 64 with rank-1 matmuls whose operands both live
    on partition 64.
  - Softmax along the free dim of L^T (scalar-engine exp with accumulated
    denominator), folded:  out = E^T @ (vbar / s) as one [64]x[64,64]
    matmul producing the output row directly.

Data-parallel over batch across the 8 NeuronCores (4 batch elements per
core); no collectives needed.
"""

import sys

import numpy as np

for _p in ("/opt/trn_rl_repo", "/root/.axon_site/_ro/trn_rl_repo"):
    if _p not in sys.path:
        sys.path.insert(0, _p)

import concourse.bacc as bacc
import concourse.mybir as mybir
import concourse.tile as tile
from concourse import masks
from concourse.bass_utils import run_bass_kernel_spmd

B, C, H, W = 32, 512, 64, 64
N = H * W            # 4096
C8 = 64              # C // 8
NCORES = 8
BPC = B // NCORES    # batch elements per core
NCHUNK = C // 128    # C chunks of 128
TW = 512             # projection tile width (PSUM bank = 512 f32)
NT = N // TW         # 8 projection tiles
NSUB = TW // 128     # transpose subtiles per projection tile

F32 = mybir.dt.float32
F32R = mybir.dt.float32r
F16 = mybir.dt.float16
AX = mybir.AxisListType.X
MULT = mybir.AluOpType.mult
ADD = mybir.AluOpType.add

_NC_CACHE = {}


def _build_nc(loop_n=None, mode="full"):
    """Build the bass program.  loop_n wraps the per-batch section in a
    device-side For_i loop (used only for timing: the NEFF then executes the
    whole workload loop_n times back-to-back, making device time measurable
    over the host dispatch overhead).  mode: "full" | "dma" (x loads only)
    | "compute" (batch-0 x loaded once outside the loop, engines only)."""
    nc = bacc.Bacc("TRN2", target_bir_lowering=False, debug=False)

    x_d = nc.dram_tensor("x", [BPC, C, N], F16, kind="ExternalInput")
    wq_d = nc.dram_tensor("wq", [C8, C], F16, kind="ExternalInput")
    bq_d = nc.dram_tensor("bq", [C8], F32, kind="ExternalInput")
    wk_d = nc.dram_tensor("wk", [C8, C], F16, kind="ExternalInput")
    bk_d = nc.dram_tensor("bk", [C8], F32, kind="ExternalInput")
    wv_d = nc.dram_tensor("wv", [C8, C], F16, kind="ExternalInput")
    bv_d = nc.dram_tensor("bv", [C8], F32, kind="ExternalInput")
    out_d = nc.dram_tensor("out", [BPC, C8], F32, kind="ExternalOutput")

    with tile.TileContext(nc, trace_sim=False) as tc:
        with (
            tc.tile_pool(name="const", bufs=1) as constp,
            tc.tile_pool(name="xpool", bufs=2) as xpool,
            tc.tile_pool(name="qkpool", bufs=4) as qkpool,
            tc.tile_pool(name="attpool", bufs=3) as attpool,
            tc.tile_pool(name="smallp", bufs=2) as smallp,
            tc.tile_pool(name="ps_qk", bufs=2, space="PSUM") as ps_qk,
            tc.tile_pool(name="ps_att", bufs=2, space="PSUM") as ps_att,
            tc.tile_pool(name="ps_small", bufs=1, space="PSUM") as ps_small,
        ):
            # ---------------- one-time prep ----------------
            ident = constp.tile([128, 128], F32)
            masks.make_identity(nc, ident[:])
            ident16 = constp.tile([128, 128], F16)
            nc.scalar.copy(ident16[:], ident[:])

            ones_row = constp.tile([1, C8], F32)
            nc.vector.memset(ones_row[:], 1.0)
            ones2_f32 = constp.tile([128, 2], F32)
            nc.vector.memset(ones2_f32[:], 1.0)
            ones2_16 = constp.tile([128, 2], F16)
            nc.scalar.copy(ones2_16[:], ones2_f32[:])
            # ones at partition 64 (to broadcast the sq row the attention
            # matmul leaves on PSUM partition 64)
            ones64 = constp.tile([C8 + 1, C8], F32)
            nc.vector.memset(ones64[C8 : C8 + 1, :], 1.0)

            wq_raw = constp.tile([C8, C], F16)
            nc.sync.dma_start(wq_raw[:], wq_d.ap()[:, :])
            wk_raw = constp.tile([C8, C], F16)
            nc.sync.dma_start(wk_raw[:], wk_d.ap()[:, :])
            wv_raw = constp.tile([C8, C], F16)
            nc.sync.dma_start(wv_raw[:], wv_d.ap()[:, :])

            bq_row = constp.tile([1, C8], F32)
            nc.sync.dma_start(bq_row[:], bq_d.ap().unsqueeze(0))
            bk_row = constp.tile([1, C8], F32)
            nc.sync.dma_start(bk_row[:], bk_d.ap().unsqueeze(0))
            bv_row = constp.tile([1, C8], F32)
            nc.sync.dma_start(bv_row[:], bv_d.ap().unsqueeze(0))

            # fused transposed weight chunks: wqkvT[c] = [wqT | wvT | wkT]
            wqkvT = []
            for c in range(NCHUNK):
                csl = slice(c * 128, (c + 1) * 128)
                pt = ps_small.tile([128, 192], F16, tag="sp")
                nc.tensor.transpose(
                    pt[:, 0:C8], wq_raw[:, csl], ident16[0:C8, 0:C8]
                )
                nc.tensor.transpose(
                    pt[:, C8 : 2 * C8], wv_raw[:, csl], ident16[0:C8, 0:C8]
                )
                nc.tensor.transpose(
                    pt[:, 2 * C8 : 192], wk_raw[:, csl], ident16[0:C8, 0:C8]
                )
                st = constp.tile([128, 192], F16, tag=f"wqkvT{c}")
                nc.scalar.copy(st[:], pt[:])
                wqkvT.append(st)

            # bias-derived constants
            p_bc = ps_small.tile([C8, C8], F32, tag="sp")
            nc.tensor.matmul(p_bc[:], ones_row[:], bq_row[:], start=True, stop=True)
            bq_bc = constp.tile([C8, C8], F32)  # every row = bq
            nc.scalar.copy(bq_bc[:], p_bc[:])

            p_bk = ps_small.tile([C8, 1], F32, tag="sp")
            nc.tensor.matmul(
                p_bk[:], bk_row[:], ones_row[:, 0:1], start=True, stop=True
            )
            bk_col = constp.tile([C8, 1], F32)
            nc.scalar.copy(bk_col[:], p_bk[:])

            p_bv = ps_small.tile([C8, 1], F32, tag="sp")
            nc.tensor.matmul(
                p_bv[:], bv_row[:], ones_row[:, 0:1], start=True, stop=True
            )
            bv_col = constp.tile([C8, 1], F32)
            nc.scalar.copy(bv_col[:], p_bv[:])

            # ---------------- per batch element ----------------
            def dma_batch_into(b, pool, tagp, nh):
                # [128, N/nh] fp16 tiles, emitted wave-major so the first
                # projection tiles' inputs land early; batch 0 uses finer
                # waves (nh=4) to shorten the cold-start bubble
                ww = N // nh
                xc = [[None] * nh for _ in range(NCHUNK)]
                for w in range(nh):
                    for c in range(NCHUNK):
                        t = pool.tile([128, ww], F16, tag=f"{tagp}{c}w{w}n{nh}")
                        nc.sync.dma_start(
                            t[:],
                            x_d.ap()[
                                b, c * 128 : (c + 1) * 128, w * ww : (w + 1) * ww
                            ],
                        )
                        xc[c][w] = t
                return xc

            xc_static = None
            if mode == "compute":
                xc_static = dma_batch_into(0, constp, "xs", 2)

            def dma_batch(b):
                return dma_batch_into(b, xpool, "x", 2)

            def emit_batches():
                if mode == "dma":
                    for b in range(BPC):
                        dma_batch(b)
                    return
                if mode == "compute":
                    fin = None
                    for b in range(BPC):
                        fin = emit_batch(b, xc_static, fin)
                    fin()
                    return
                xc_next = dma_batch(0)
                fin = None
                for b in range(BPC):
                    xc_cur = xc_next
                    if b + 1 < BPC:
                        xc_next = dma_batch(b + 1)
                    fin = emit_batch(b, xc_cur, fin)
                fin()

            def emit_attn(att_ps, ti, a_list):
                for s in range(NSUB):
                    first = ti == 0 and s == 0
                    last = ti == NT - 1 and s == NSUB - 1
                    # lhsT=[kT|1], rhs=[1|qT] -> out[65,65]:
                    #   [0:64, 0] = sk, [0:64, 1:65] = L0T,
                    #   [64, 1:65] = sq, [64, 0] = N
                    nc.tensor.matmul(
                        att_ps[:],
                        a_list[:, s, 129:194],
                        a_list[:, s, 0:129],
                        start=first,
                        stop=last,
                    )

            def emit_batch(b, xc, fin_prev):
                pending = None

                # [65, 129]: [0:64,0]=sk, [0:64,1:65]=L0T, [64,1:65]=sq,
                # [64,65:129]=sum_n v0 (accumulated over all subtiles)
                att_ps = ps_att.tile([C8 + 1, 2 * C8 + 1], F32)

                nh = len(xc[0])
                for ti in range(NT):
                    hh = ti // (NT // nh)
                    base = (ti % (NT // nh)) * TW
                    sl = slice(base, base + TW)
                    # x-stationary fused projection: one matmul per
                    # (chunk, subtile) streams [wqT | wvT | wkT] through the
                    # stationary x chunk -> [qT | vT | kT] in n-major layout
                    qk_ps = ps_qk.tile([128, NSUB, 256], F32, tag="qk_ps")
                    for s in range(NSUB):
                        nsl = slice(base + s * 128, base + (s + 1) * 128)
                        for c in range(NCHUNK):
                            nc.tensor.matmul(
                                qk_ps[:, s, 0:192],
                                xc[c][hh][:, nsl],
                                wqkvT[c][:],
                                start=(c == 0),
                                stop=(c == NCHUNK - 1),
                            )

                    # one [128, 4x194] tile holds all 4 subtiles' attention
                    # operands [ones | qT | vT | kT | ones]; single 3D copies
                    a_sb = attpool.tile([128, NSUB, 194], F16, tag="a_sb")
                    nc.vector.tensor_copy(a_sb[:, :, 1:193], qk_ps[:, :, 0:192])
                    nc.vector.tensor_copy(
                        a_sb[:, :, 0:194:193],
                        ones2_16[:].unsqueeze(1).broadcast_to([128, NSUB, 2]),
                    )
                    a_list = a_sb
                    # attention matmuls run one projection tile behind, so
                    # their a_sb inputs were copied a whole tile ago (no PE
                    # stall on the DVE copy)
                    if pending is not None:
                        emit_attn(*pending)
                    pending = (att_ps, ti, a_list)
                    if ti == 1 and fin_prev is not None:
                        # previous batch's finalize chain runs here: its
                        # inputs completed a full tile ago, so the PE ops
                        # inside it never stall the engine
                        fin_prev()

                if pending is not None:
                    emit_attn(*pending)
                    pending = None

                # DVE/ACT precursors right after the attention flush; the
                # PE-bearing remainder is deferred one tile into the next
                # batch so its operands are long since ready
                skp = smallp.tile([C8, 1], F32, tag="skp")
                nc.vector.scalar_tensor_tensor(
                    skp[:], bk_col[:], float(N), att_ps[0:C8, 0:1], op0=MULT, op1=ADD
                )
                sq_sb = smallp.tile([C8 + 1, C8], F32, tag="sq_sb")
                nc.scalar.copy(sq_sb[C8 : C8 + 1, :], att_ps[C8 : C8 + 1, 1 : C8 + 1])
                # v sums sit on partition 64, cols 65:129
                vrow_sb = smallp.tile([C8 + 1, C8], F32, tag="vrow_sb")
                nc.scalar.copy(
                    vrow_sb[C8 : C8 + 1, :], att_ps[C8 : C8 + 1, C8 + 1 : 2 * C8 + 1]
                )

                return lambda: finalize_batch(b, att_ps, skp, sq_sb, vrow_sb)

            def finalize_batch(b, att_ps, skp, sq_sb, vrow_sb):
                # vsum row (partition 64) -> column via rank-1 matmul at p64
                vb_ps = ps_small.tile([C8, 1], F32, tag="sp")
                nc.tensor.matmul(
                    vb_ps[:],
                    vrow_sb[C8 : C8 + 1, :],
                    ones64[C8 : C8 + 1, 0:1],
                    start=True,
                    stop=True,
                )
                vbar = smallp.tile([C8, 1], F32, tag="vbar")
                nc.vector.scalar_tensor_tensor(
                    vbar[:], vb_ps[:], 1.0 / N, bv_col[:], op0=MULT, op1=ADD
                )
                # broadcast sq (row on partition 64) to all partitions
                sq_ps = ps_small.tile([C8, C8], F32, tag="sp")
                nc.tensor.matmul(
                    sq_ps[:],
                    ones64[C8 : C8 + 1, :],
                    sq_sb[C8 : C8 + 1, :],
                    start=True,
                    stop=True,
                )
                # LT = L0T + bq_bc * skp + sq_bc * bk
                L1 = smallp.tile([C8, C8], F32, tag="L1")
                nc.vector.scalar_tensor_tensor(
                    L1[:], bq_bc[:], skp[:], att_ps[0:C8, 1 : C8 + 1],
                    op0=MULT, op1=ADD,
                )
                LT = smallp.tile([C8, C8], F32, tag="LT")
                nc.vector.scalar_tensor_tensor(
                    LT[:], sq_ps[:], bk_col[:], L1[:], op0=MULT, op1=ADD
                )
                # softmax along free dim (the o axis)
                negm = smallp.tile([C8, 1], F32, tag="negm")
                nc.vector.reduce_max(negm[:], LT[:], axis=AX, negate=True)
                E = smallp.tile([C8, C8], F32, tag="E")
                s_col = smallp.tile([C8, 1], F32, tag="s_col")
                nc.scalar.activation(
                    E[:],
                    LT[:],
                    mybir.ActivationFunctionType.Exp,
                    bias=negm[:],
                    scale=1.0,
                    accum_out=s_col[:],
                )
                # w = vbar / s ; out = E^T @ w  (as row via lhsT=w)
                rs = smallp.tile([C8, 1], F32, tag="rs")
                nc.vector.reciprocal(rs[:], s_col[:])
                wcol = smallp.tile([C8, 1], F32, tag="wcol")
                nc.vector.tensor_tensor(wcol[:], vbar[:], rs[:], op=MULT)
                out_ps = ps_small.tile([1, C8], F32, tag="sp")
                nc.tensor.matmul(out_ps[:], wcol[:], E[:], start=True, stop=True)
                out_row = smallp.tile([1, C8], F32, tag="out_row")
                nc.scalar.copy(out_row[:], out_ps[:])
                nc.gpsimd.dma_start(out_d.ap()[b : b + 1, :], out_row[:])

            if loop_n is None:
                emit_batches()
            else:
                hints = (
                    mybir.EngineType.PE,
                    mybir.EngineType.DVE,
                    mybir.EngineType.Activation,
                    mybir.EngineType.SP,
                    mybir.EngineType.Pool,
                )
                with tc.For_i(0, loop_n, 1, hint_engines=hints):
                    emit_batches()

    nc.compile()
    return nc


def _get_nc(loop_n=None, mode="full"):
    key = ("nc", loop_n, mode)
    if key not in _NC_CACHE:
        _NC_CACHE[key] = _build_nc(loop_n, mode)
    return _NC_CACHE[key]


def _make_in_maps(x, wq, bq, wk, bk, wv, bv):
    # fp16 shipping: same 10-bit mantissa as the tf32-class device compute,
    # but halves the HBM traffic for x
    xf = np.ascontiguousarray(
        np.asarray(x, dtype=np.float32).reshape(B, C, N).astype(np.float16)
    )
    shared = {
        "wq": np.asarray(wq, np.float32).astype(np.float16),
        "bq": np.asarray(bq, np.float32),
        "wk": np.asarray(wk, np.float32).astype(np.float16),
        "bk": np.asarray(bk, np.float32),
        "wv": np.asarray(wv, np.float32).astype(np.float16),
        "bv": np.asarray(bv, np.float32),
    }
    return [
        {"x": xf[i * BPC : (i + 1) * BPC], **shared} for i in range(NCORES)
    ]


def kernel(x, wq, bq, wk, bk, wv, bv):
    nc = _get_nc()
    in_maps = _make_in_maps(x, wq, bq, wk, bk, wv, bv)
    res = run_bass_kernel_spmd(nc, in_maps, core_ids=list(range(NCORES)))
    out = np.concatenate([res.results[i]["out"] for i in range(NCORES)], axis=0)
    return out.astype(np.float32)



# revision 2
# speedup vs baseline: 1.0035x; 1.0035x over previous
"""Trainium2 Bass kernel for AttentionPooling.

Math (per batch element b):
  xf = x[b] reshaped [C, N] with C=512, N=4096
  q = wq@xf + bq ; k = wk@xf + bk ; v = wv@xf + bv          (each [64, N])
  logits = q @ k^T  [64, 64];  attn = softmax(logits, axis over rows o)
  out[b] = mean_n(attn @ v) = attn @ mean_n(v)              ([64])

Because attn does not depend on n, mean_n(attn @ v) = attn @ vbar with
vbar = mean_n(v) — the heavy [64, N] attn@v product collapses to a [64]
vector, so only the q/k projections and a 64x64 logits product are real
work.  Implementation, per batch element:

  - x is shipped as fp16 (10-bit mantissa, same class as tf32/f32r
    device rounding; empirically 4.5e-3 end-to-end rel err) which halves
    the HBM traffic for x — the dominant memory cost.
  - x[b] arrives in ONE 4MB dma_start into a [128, 4, 4096] SBUF tile
    via the AP view (p j) n -> p j n: partition p holds channels
    4p..4p+3, i.e. 32KB contiguous DRAM per partition -> 128 fat
    descriptors, near-peak HBM bandwidth.  The channel permutation
    c = 4p + j is folded into the host-side weight layout.
  - X-STATIONARY fused projection: each [128, 128] x chunk-subtile is
    the matmul weight; ONE matmul per (chunk, subtile) streams the fused
    [wqT | wvT | wkT] 192-column block through it, producing
    [q0T | v0T | k0T] directly in n-major layout (no transposes, half
    the PE instructions), fp32 PSUM accumulation over the 4 C-chunks.
    The weight block is pre-transposed AND channel-permuted on the host
    and shipped as one [128, 4, 192] fp16 tensor.
  - One 3D-AP vector-engine copy per projection tile moves all 4
    subtiles' [ones | qT | vT | kT | ones] attention operands to SBUF;
    the ones-augmented [65, 129] attention matmuls (lhsT=[kT|1],
    rhs=[1|qT|vT], accumulated over all 32 n-subtiles) produce L0^T AND
    sum_n k0 (column 0), sum_n q0, sum_n v0 (partition-64 row) — the
    bias-correction and pooling sums ride the same accumulation.  They
    run one projection tile behind so they never stall on the copies.
  - Bias corrections applied analytically on the 64x64 logits:
      L^T = L0^T + bq (x) (sk + N bk) + bk (x) sq
    (valid because logits(q0+bq, k0+bk) is bilinear and attn/softmax only
    needs the full L^T).  sq is broadcast and the v-sum row transposed
    off PSUM partition 64 with rank-1 matmuls whose operands both live
    on partition 64.
  - Softmax along the free dim of L^T (scalar-engine exp with accumulated
    denominator), folded:  out = E^T @ (vbar / s) as one [64]x[64,64]
    matmul producing the output row directly.

Data-parallel over batch across the 8 NeuronCores (4 batch elements per
core); no collectives needed.
"""

import sys

import numpy as np

for _p in ("/opt/trn_rl_repo", "/root/.axon_site/_ro/trn_rl_repo"):
    if _p not in sys.path:
        sys.path.insert(0, _p)

import concourse.bacc as bacc
import concourse.mybir as mybir
import concourse.tile as tile
from concourse.bass_utils import run_bass_kernel_spmd

B, C, H, W = 32, 512, 64, 64
N = H * W            # 4096
C8 = 64              # C // 8
NCORES = 8
BPC = B // NCORES    # batch elements per core
NCHUNK = C // 128    # C chunks of 128
TW = 512             # projection tile width (PSUM bank = 512 f32)
NT = N // TW         # 8 projection tiles
NSUB = TW // 128     # transpose subtiles per projection tile

F32 = mybir.dt.float32
F16 = mybir.dt.float16
AX = mybir.AxisListType.X
MULT = mybir.AluOpType.mult
ADD = mybir.AluOpType.add

_NC_CACHE = {}


def _build_nc(loop_n=None, mode="full"):
    """Build the bass program.  loop_n wraps the per-batch section in a
    device-side For_i loop (used only for timing: the NEFF then executes the
    whole workload loop_n times back-to-back, making device time measurable
    over the host dispatch overhead).  mode: "full" | "dma" (x loads only)
    | "compute" (batch-0 x loaded once outside the loop, engines only)."""
    nc = bacc.Bacc("TRN2", target_bir_lowering=False, debug=False)

    x_d = nc.dram_tensor("x", [BPC, C, N], F16, kind="ExternalInput")
    wqkvT_d = nc.dram_tensor("wqkvT", [128, NCHUNK, 192], F16, kind="ExternalInput")
    bq_d = nc.dram_tensor("bq", [C8], F32, kind="ExternalInput")
    bk_d = nc.dram_tensor("bk", [C8], F32, kind="ExternalInput")
    bv_d = nc.dram_tensor("bv", [C8], F32, kind="ExternalInput")
    out_d = nc.dram_tensor("out", [BPC, C8], F32, kind="ExternalOutput")

    with tile.TileContext(nc, trace_sim=False) as tc:
        with (
            tc.tile_pool(name="const", bufs=1) as constp,
            tc.tile_pool(name="xpool", bufs=2) as xpool,
            tc.tile_pool(name="attpool", bufs=3) as attpool,
            tc.tile_pool(name="smallp", bufs=2) as smallp,
            tc.tile_pool(name="ps_qk", bufs=2, space="PSUM") as ps_qk,
            tc.tile_pool(name="ps_att", bufs=2, space="PSUM") as ps_att,
            tc.tile_pool(name="ps_small", bufs=1, space="PSUM") as ps_small,
        ):
            # ---------------- one-time prep ----------------
            ones_row = constp.tile([1, C8], F32)
            nc.vector.memset(ones_row[:], 1.0)
            ones2_f32 = constp.tile([128, 2], F32)
            nc.vector.memset(ones2_f32[:], 1.0)
            ones2_16 = constp.tile([128, 2], F16)
            nc.scalar.copy(ones2_16[:], ones2_f32[:])
            # ones at partition 64 (to broadcast the sq row the attention
            # matmul leaves on PSUM partition 64)
            ones64 = constp.tile([C8 + 1, C8], F32)
            nc.vector.memset(ones64[C8 : C8 + 1, :], 1.0)

            # pre-transposed, channel-permuted fused weights [p, chunk, 192]
            wsb = constp.tile([128, NCHUNK, 192], F16)
            nc.sync.dma_start(wsb[:], wqkvT_d.ap()[:, :, :])

            bq_row = constp.tile([1, C8], F32)
            nc.sync.dma_start(bq_row[:], bq_d.ap().unsqueeze(0))
            bk_row = constp.tile([1, C8], F32)
            nc.sync.dma_start(bk_row[:], bk_d.ap().unsqueeze(0))
            bv_row = constp.tile([1, C8], F32)
            nc.sync.dma_start(bv_row[:], bv_d.ap().unsqueeze(0))

            # bias-derived constants
            p_bc = ps_small.tile([C8, C8], F32, tag="sp")
            nc.tensor.matmul(p_bc[:], ones_row[:], bq_row[:], start=True, stop=True)
            bq_bc = constp.tile([C8, C8], F32)  # every row = bq
            nc.scalar.copy(bq_bc[:], p_bc[:])

            p_bk = ps_small.tile([C8, 1], F32, tag="sp")
            nc.tensor.matmul(
                p_bk[:], bk_row[:], ones_row[:, 0:1], start=True, stop=True
            )
            bk_col = constp.tile([C8, 1], F32)
            nc.scalar.copy(bk_col[:], p_bk[:])

            p_bv = ps_small.tile([C8, 1], F32, tag="sp")
            nc.tensor.matmul(
                p_bv[:], bv_row[:], ones_row[:, 0:1], start=True, stop=True
            )
            bv_col = constp.tile([C8, 1], F32)
            nc.scalar.copy(bv_col[:], p_bv[:])

            # ---------------- per batch element ----------------
            def dma_batch_into(b, pool, tagp):
                # one 4MB dma_start: [128, 4, 4096] fp16, partition p holds
                # channels 4p..4p+3 => 32KB contiguous DRAM per partition
                t = pool.tile([128, NCHUNK, N], F16, tag=f"{tagp}")
                nc.sync.dma_start(
                    t[:], x_d.ap()[b].rearrange("(p j) n -> p j n", j=NCHUNK)
                )
                return t

            xc_static = None
            if mode == "compute":
                xc_static = dma_batch_into(0, constp, "xs")

            def dma_batch(b):
                return dma_batch_into(b, xpool, "x")

            def emit_batches():
                if mode == "dma":
                    for b in range(BPC):
                        dma_batch(b)
                    return
                if mode == "compute":
                    fin = None
                    for b in range(BPC):
                        fin = emit_batch(b, xc_static, fin)
                    fin()
                    return
                xc_next = dma_batch(0)
                fin = None
                for b in range(BPC):
                    xc_cur = xc_next
                    if b + 1 < BPC:
                        xc_next = dma_batch(b + 1)
                    fin = emit_batch(b, xc_cur, fin)
                fin()

            def emit_attn(att_ps, ti, a_list):
                for s in range(NSUB):
                    first = ti == 0 and s == 0
                    last = ti == NT - 1 and s == NSUB - 1
                    # lhsT=[kT|1], rhs=[1|qT] -> out[65,65]:
                    #   [0:64, 0] = sk, [0:64, 1:65] = L0T,
                    #   [64, 1:65] = sq, [64, 0] = N
                    nc.tensor.matmul(
                        att_ps[:],
                        a_list[:, s, 129:194],
                        a_list[:, s, 0:129],
                        start=first,
                        stop=last,
                    )

            def emit_batch(b, xt, fin_prev):
                pending = None

                # [65, 129]: [0:64,0]=sk, [0:64,1:65]=L0T, [64,1:65]=sq,
                # [64,65:129]=sum_n v0 (accumulated over all subtiles)
                att_ps = ps_att.tile([C8 + 1, 2 * C8 + 1], F32)

                for ti in range(NT):
                    base = ti * TW
                    # x-stationary fused projection: one matmul per
                    # (chunk, subtile) streams [wqT | wvT | wkT] through the
                    # stationary x chunk -> [qT | vT | kT] in n-major layout
                    qk_ps = ps_qk.tile([128, NSUB, 256], F32, tag="qk_ps")
                    for s in range(NSUB):
                        nsl = slice(base + s * 128, base + (s + 1) * 128)
                        for j in range(NCHUNK):
                            nc.tensor.matmul(
                                qk_ps[:, s, 0:192],
                                xt[:, j, nsl],
                                wsb[:, j, :],
                                start=(j == 0),
                                stop=(j == NCHUNK - 1),
                            )

                    # one [128, 4x194] tile holds all 4 subtiles' attention
                    # operands [ones | qT | vT | kT | ones]; single 3D copies
                    a_sb = attpool.tile([128, NSUB, 194], F16, tag="a_sb")
                    nc.vector.tensor_copy(a_sb[:, :, 1:193], qk_ps[:, :, 0:192])
                    nc.vector.tensor_copy(
                        a_sb[:, :, 0:194:193],
                        ones2_16[:].unsqueeze(1).broadcast_to([128, NSUB, 2]),
                    )
                    a_list = a_sb
                    # attention matmuls run one projection tile behind, so
                    # their a_sb inputs were copied a whole tile ago (no PE
                    # stall on the DVE copy)
                    if pending is not None:
                        emit_attn(*pending)
                    pending = (att_ps, ti, a_list)
                    if ti == 1 and fin_prev is not None:
                        # previous batch's finalize chain runs here: its
                        # inputs completed a full tile ago, so the PE ops
                        # inside it never stall the engine
                        fin_prev()

                if pending is not None:
                    emit_attn(*pending)
                    pending = None

                # DVE/ACT precursors right after the attention flush; the
                # PE-bearing remainder is deferred one tile into the next
                # batch so its operands are long since ready
                skp = smallp.tile([C8, 1], F32, tag="skp")
                nc.vector.scalar_tensor_tensor(
                    skp[:], bk_col[:], float(N), att_ps[0:C8, 0:1], op0=MULT, op1=ADD
                )
                sq_sb = smallp.tile([C8 + 1, C8], F32, tag="sq_sb")
                nc.scalar.copy(sq_sb[C8 : C8 + 1, :], att_ps[C8 : C8 + 1, 1 : C8 + 1])
                # v sums sit on partition 64, cols 65:129
                vrow_sb = smallp.tile([C8 + 1, C8], F32, tag="vrow_sb")
                nc.scalar.copy(
                    vrow_sb[C8 : C8 + 1, :], att_ps[C8 : C8 + 1, C8 + 1 : 2 * C8 + 1]
                )

                return lambda: finalize_batch(b, att_ps, skp, sq_sb, vrow_sb)

            def finalize_batch(b, att_ps, skp, sq_sb, vrow_sb):
                # vsum row (partition 64) -> column via rank-1 matmul at p64
                vb_ps = ps_small.tile([C8, 1], F32, tag="sp")
                nc.tensor.matmul(
                    vb_ps[:],
                    vrow_sb[C8 : C8 + 1, :],
                    ones64[C8 : C8 + 1, 0:1],
                    start=True,
                    stop=True,
                )
                vbar = smallp.tile([C8, 1], F32, tag="vbar")
                nc.vector.scalar_tensor_tensor(
                    vbar[:], vb_ps[:], 1.0 / N, bv_col[:], op0=MULT, op1=ADD
                )
                # broadcast sq (row on partition 64) to all partitions
                sq_ps = ps_small.tile([C8, C8], F32, tag="sp")
                nc.tensor.matmul(
                    sq_ps[:],
                    ones64[C8 : C8 + 1, :],
                    sq_sb[C8 : C8 + 1, :],
                    start=True,
                    stop=True,
                )
                # LT = L0T + bq_bc * skp + sq_bc * bk
                L1 = smallp.tile([C8, C8], F32, tag="L1")
                nc.vector.scalar_tensor_tensor(
                    L1[:], bq_bc[:], skp[:], att_ps[0:C8, 1 : C8 + 1],
                    op0=MULT, op1=ADD,
                )
                LT = smallp.tile([C8, C8], F32, tag="LT")
                nc.vector.scalar_tensor_tensor(
                    LT[:], sq_ps[:], bk_col[:], L1[:], op0=MULT, op1=ADD
                )
                # softmax along free dim (the o axis)
                negm = smallp.tile([C8, 1], F32, tag="negm")
                nc.vector.reduce_max(negm[:], LT[:], axis=AX, negate=True)
                E = smallp.tile([C8, C8], F32, tag="E")
                s_col = smallp.tile([C8, 1], F32, tag="s_col")
                nc.scalar.activation(
                    E[:],
                    LT[:],
                    mybir.ActivationFunctionType.Exp,
                    bias=negm[:],
                    scale=1.0,
                    accum_out=s_col[:],
                )
                # w = vbar / s ; out = E^T @ w  (as row via lhsT=w)
                rs = smallp.tile([C8, 1], F32, tag="rs")
                nc.vector.reciprocal(rs[:], s_col[:])
                wcol = smallp.tile([C8, 1], F32, tag="wcol")
                nc.vector.tensor_tensor(wcol[:], vbar[:], rs[:], op=MULT)
                out_ps = ps_small.tile([1, C8], F32, tag="sp")
                nc.tensor.matmul(out_ps[:], wcol[:], E[:], start=True, stop=True)
                out_row = smallp.tile([1, C8], F32, tag="out_row")
                nc.scalar.copy(out_row[:], out_ps[:])
                nc.gpsimd.dma_start(out_d.ap()[b : b + 1, :], out_row[:])

            if loop_n is None:
                emit_batches()
            else:
                hints = (
                    mybir.EngineType.PE,
                    mybir.EngineType.DVE,
                    mybir.EngineType.Activation,
                    mybir.EngineType.SP,
                    mybir.EngineType.Pool,
                )
                with tc.For_i(0, loop_n, 1, hint_engines=hints):
                    emit_batches()

    nc.compile()
    return nc


def _get_nc(loop_n=None, mode="full"):
    key = ("nc", loop_n, mode)
    if key not in _NC_CACHE:
        _NC_CACHE[key] = _build_nc(loop_n, mode)
    return _NC_CACHE[key]


def _make_in_maps(x, wq, bq, wk, bk, wv, bv):
    # fp16 shipping: same 10-bit mantissa as the tf32-class device compute,
    # but halves the HBM traffic for x
    xf = np.ascontiguousarray(
        np.asarray(x, dtype=np.float32).reshape(B, C, N).astype(np.float16)
    )
    # fused [wq | wv | wk] block, transposed to [C, 192] then viewed as
    # [128, 4, 192]: row c = 4p + j lands at (p, j) — matching the device's
    # (p j) n -> p j n view of x
    wall = np.concatenate(
        [
            np.asarray(wq, np.float32),
            np.asarray(wv, np.float32),
            np.asarray(wk, np.float32),
        ],
        axis=0,
    )  # [192, C]
    wqkvT = np.ascontiguousarray(
        wall.T.reshape(128, NCHUNK, 192).astype(np.float16)
    )
    shared = {
        "wqkvT": wqkvT,
        "bq": np.asarray(bq, np.float32),
        "bk": np.asarray(bk, np.float32),
        "bv": np.asarray(bv, np.float32),
    }
    return [
        {"x": xf[i * BPC : (i + 1) * BPC], **shared} for i in range(NCORES)
    ]


def kernel(x, wq, bq, wk, bk, wv, bv):
    nc = _get_nc()
    in_maps = _make_in_maps(x, wq, bq, wk, bk, wv, bv)
    res = run_bass_kernel_spmd(nc, in_maps, core_ids=list(range(NCORES)))
    out = np.concatenate([res.results[i]["out"] for i in range(NCORES)], axis=0)
    return out.astype(np.float32)


# revision 12
# speedup vs baseline: 1.0863x; 1.0825x over previous
"""Trainium2 Bass kernel for AttentionPooling.

Math (per batch element b):
  xf = x[b] reshaped [C, N] with C=512, N=4096
  q = wq@xf + bq ; k = wk@xf + bk ; v = wv@xf + bv          (each [64, N])
  logits = q @ k^T  [64, 64];  attn = softmax(logits, axis over rows o)
  out[b] = mean_n(attn @ v) = attn @ mean_n(v)              ([64])

Because attn does not depend on n, mean_n(attn @ v) = attn @ vbar with
vbar = mean_n(v) — the heavy [64, N] attn@v product collapses to a [64]
vector, so only the q/k projections and a 64x64 logits product are real
work.  Implementation, per batch element:

  - x is shipped as fp16 (10-bit mantissa, same class as tf32/f32r
    device rounding; empirically 4.5e-3 end-to-end rel err) which halves
    the HBM traffic for x — the dominant memory cost.
  - x[b] arrives in ONE 4MB dma_start into a [128, 4, 4096] SBUF tile
    via the AP view (p j) n -> p j n: partition p holds channels
    4p..4p+3, i.e. 32KB contiguous DRAM per partition -> 128 fat
    descriptors, near-peak HBM bandwidth.  The channel permutation
    c = 4p + j is folded into the host-side weight layout.
  - X-STATIONARY fused projection: each [128, 128] x chunk-subtile is
    the matmul weight; ONE matmul per (chunk, subtile) streams the fused
    [wqT | wvT | wkT] 192-column block through it, producing
    [q0T | v0T | k0T] directly in n-major layout (no transposes, half
    the PE instructions), fp32 PSUM accumulation over the 4 C-chunks.
    The weight block is pre-transposed AND channel-permuted on the host
    and shipped as one [128, 4, 192] fp16 tensor.
  - One 3D-AP vector-engine copy per projection tile moves all 4
    subtiles' [ones | qT | vT | kT | ones] attention operands to SBUF;
    the ones-augmented [65, 129] attention matmuls (lhsT=[kT|1],
    rhs=[1|qT|vT], accumulated over all 32 n-subtiles) produce L0^T AND
    sum_n k0 (column 0), sum_n q0, sum_n v0 (partition-64 row) — the
    bias-correction and pooling sums ride the same accumulation.  They
    run one projection tile behind so they never stall on the copies.
  - Bias corrections applied analytically on the 64x64 logits:
      L^T = L0^T + bq (x) (sk + N bk) + bk (x) sq
    (valid because logits(q0+bq, k0+bk) is bilinear and attn/softmax only
    needs the full L^T).  sq is broadcast and the v-sum row transposed
    off PSUM partition 64 with rank-1 matmuls whose operands both live
    on partition 64.
  - Softmax along the free dim of L^T (scalar-engine exp with accumulated
    denominator), folded:  out = E^T @ (vbar / s) as one [64]x[64,64]
    matmul producing the output row directly.

Data-parallel over batch across the 8 NeuronCores (4 batch elements per
core); no collectives needed.
"""

import sys

import numpy as np

for _p in ("/opt/trn_rl_repo", "/root/.axon_site/_ro/trn_rl_repo"):
    if _p not in sys.path:
        sys.path.insert(0, _p)

import concourse.bacc as bacc
import concourse.mybir as mybir
import concourse.tile as tile
from concourse.bass_utils import run_bass_kernel_spmd

B, C, H, W = 32, 512, 64, 64
N = H * W            # 4096
C8 = 64              # C // 8
NCORES = 8
BPC = B // NCORES    # batch elements per core
NCHUNK = C // 128    # C chunks of 128
TW = 512             # projection tile width (PSUM bank = 512 f32)
NT = N // TW         # 8 projection tiles
NSUB = TW // 128     # transpose subtiles per projection tile

F32 = mybir.dt.float32
F16 = mybir.dt.float16
AX = mybir.AxisListType.X
MULT = mybir.AluOpType.mult
ADD = mybir.AluOpType.add

_NC_CACHE = {}


def _build_nc(loop_n=None, mode="full"):
    """Build the bass program.  loop_n wraps the per-batch section in a
    device-side For_i loop (used only for timing: the NEFF then executes the
    whole workload loop_n times back-to-back, making device time measurable
    over the host dispatch overhead).  mode: "full" | "dma" (x loads only)
    | "compute" (batch-0 x loaded once outside the loop, engines only)."""
    nc = bacc.Bacc("TRN2", target_bir_lowering=False, debug=False)

    x_d = nc.dram_tensor("x", [BPC, C, N], F16, kind="ExternalInput")
    wqkvT_d = nc.dram_tensor("wqkvT", [128, NCHUNK, 192], F16, kind="ExternalInput")
    bq_d = nc.dram_tensor("bq", [C8], F32, kind="ExternalInput")
    bk_d = nc.dram_tensor("bk", [C8], F32, kind="ExternalInput")
    bv_d = nc.dram_tensor("bv", [C8], F32, kind="ExternalInput")
    out_d = nc.dram_tensor("out", [BPC, C8], F32, kind="ExternalOutput")

    with tile.TileContext(nc, trace_sim=False) as tc:
        with (
            tc.tile_pool(name="const", bufs=1) as constp,
            tc.tile_pool(name="xpool", bufs=2) as xpool,
            tc.tile_pool(name="attpool", bufs=3) as attpool,
            tc.tile_pool(name="smallp", bufs=2) as smallp,
            tc.tile_pool(name="ps_qk", bufs=2, space="PSUM") as ps_qk,
            tc.tile_pool(name="ps_att", bufs=2, space="PSUM") as ps_att,
            tc.tile_pool(name="ps_small", bufs=1, space="PSUM") as ps_small,
        ):
            # ---------------- one-time prep ----------------
            ones_row = constp.tile([1, C8], F32)
            nc.vector.memset(ones_row[:], 1.0)
            ones2_f32 = constp.tile([128, 2], F32)
            nc.vector.memset(ones2_f32[:], 1.0)
            ones2_16 = constp.tile([128, 2], F16)
            nc.scalar.copy(ones2_16[:], ones2_f32[:])
            # ones at partition 64 (to broadcast the sq row the attention
            # matmul leaves on PSUM partition 64)
            ones64 = constp.tile([C8 + 1, C8], F32)
            nc.vector.memset(ones64[C8 : C8 + 1, :], 1.0)

            # pre-transposed, channel-permuted fused weights [p, chunk, 192]
            wsb = constp.tile([128, NCHUNK, 192], F16)
            nc.sync.dma_start(wsb[:], wqkvT_d.ap()[:, :, :])

            bq_row = constp.tile([1, C8], F32)
            nc.sync.dma_start(bq_row[:], bq_d.ap().unsqueeze(0))
            bk_row = constp.tile([1, C8], F32)
            nc.sync.dma_start(bk_row[:], bk_d.ap().unsqueeze(0))
            bv_row = constp.tile([1, C8], F32)
            nc.sync.dma_start(bv_row[:], bv_d.ap().unsqueeze(0))

            # bias-derived constants
            p_bc = ps_small.tile([C8, C8], F32, tag="sp")
            nc.tensor.matmul(p_bc[:], ones_row[:], bq_row[:], start=True, stop=True)
            bq_bc = constp.tile([C8, C8], F32)  # every row = bq
            nc.scalar.copy(bq_bc[:], p_bc[:])

            p_bk = ps_small.tile([C8, 1], F32, tag="sp")
            nc.tensor.matmul(
                p_bk[:], bk_row[:], ones_row[:, 0:1], start=True, stop=True
            )
            bk_col = constp.tile([C8, 1], F32)
            nc.scalar.copy(bk_col[:], p_bk[:])

            p_bv = ps_small.tile([C8, 1], F32, tag="sp")
            nc.tensor.matmul(
                p_bv[:], bv_row[:], ones_row[:, 0:1], start=True, stop=True
            )
            bv_col = constp.tile([C8, 1], F32)
            nc.scalar.copy(bv_col[:], p_bv[:])

            # ---------------- per batch element ----------------
            XSPLIT = 4

            def dma_batch_into(b, pool, tagp, nsplit=XSPLIT):
                # [128, 4, N/nsplit] fp16 tiles; partition p holds channels
                # 4p..4p+3.  The For_i timing loop drains the whole pipeline
                # at each iteration boundary, so x moves as uniform 1MB
                # pieces in strict consumption order: the exposed head DMA
                # is one piece, and no big transfer head-of-line-blocks a
                # piece needed sooner.  Pieces alternate between the SP and
                # ACT HWDGE rings so the two rings ping-pong.
                w = N // nsplit
                ts = []
                for i in range(nsplit):
                    t = pool.tile([128, NCHUNK, w], F16, tag=f"{tagp}s{i}n{nsplit}")
                    eng = nc.sync if (b * nsplit + i) % 2 == 0 else nc.scalar
                    eng.dma_start(
                        t[:],
                        x_d.ap()[b].rearrange("(p j) n -> p j n", j=NCHUNK)[
                            :, :, i * w : (i + 1) * w
                        ],
                    )
                    ts.append(t)
                return (w, ts)

            xc_static = None
            if mode == "compute":
                xc_static = dma_batch_into(0, constp, "xs")

            def dma_batch(b):
                return dma_batch_into(b, xpool, "x")

            def emit_batches():
                if mode == "dma":
                    for b in range(BPC):
                        dma_batch(b)
                    return
                if mode == "compute":
                    fin = None
                    for b in range(BPC):
                        fin = emit_batch(b, xc_static, fin)
                    fin()
                    return
                xc_next = dma_batch(0)
                fin = None
                for b in range(BPC):
                    xc_cur = xc_next
                    if b + 1 < BPC:
                        xc_next = dma_batch(b + 1)
                    fin = emit_batch(b, xc_cur, fin)
                fin()

            def emit_attn(att_ps, ti, a_list):
                for s in range(NSUB):
                    first = ti == 0 and s == 0
                    last = ti == NT - 1 and s == NSUB - 1
                    # lhsT=[kT|1], rhs=[1|qT] -> out[65,65]:
                    #   [0:64, 0] = sk, [0:64, 1:65] = L0T,
                    #   [64, 1:65] = sq, [64, 0] = N
                    nc.tensor.matmul(
                        att_ps[:],
                        a_list[:, s, 129:194],
                        a_list[:, s, 0:129],
                        start=first,
                        stop=last,
                    )

            def emit_batch(b, xt, fin_prev):
                pending = None

                # [65, 129]: [0:64,0]=sk, [0:64,1:65]=L0T, [64,1:65]=sq,
                # [64,65:129]=sum_n v0 (accumulated over all subtiles)
                att_ps = ps_att.tile([C8 + 1, 2 * C8 + 1], F32)

                xw, xts = xt
                for ti in range(NT):
                    base = ti * TW
                    # x-stationary fused projection: one matmul per
                    # (chunk, subtile) streams [wqT | wvT | wkT] through the
                    # stationary x chunk -> [qT | vT | kT] in n-major layout
                    qk_ps = ps_qk.tile([128, NSUB, 256], F32, tag="qk_ps")
                    for s in range(NSUB):
                        n0 = base + s * 128
                        xti = xts[n0 // xw]
                        nsl = slice(n0 % xw, n0 % xw + 128)
                        for j in range(NCHUNK):
                            nc.tensor.matmul(
                                qk_ps[:, s, 0:192],
                                xti[:, j, nsl],
                                wsb[:, j, :],
                                start=(j == 0),
                                stop=(j == NCHUNK - 1),
                            )

                    # one [128, 4x194] tile holds all 4 subtiles' attention
                    # operands [ones | qT | vT | kT | ones]; single 3D copies
                    a_sb = attpool.tile([128, NSUB, 194], F16, tag="a_sb")
                    nc.vector.tensor_copy(a_sb[:, :, 1:193], qk_ps[:, :, 0:192])
                    nc.vector.tensor_copy(
                        a_sb[:, :, 0:194:193],
                        ones2_16[:].unsqueeze(1).broadcast_to([128, NSUB, 2]),
                    )
                    a_list = a_sb
                    # attention matmuls run one projection tile behind, so
                    # their a_sb inputs were copied a whole tile ago (no PE
                    # stall on the DVE copy)
                    if pending is not None:
                        emit_attn(*pending)
                    pending = (att_ps, ti, a_list)
                    if ti == 1 and fin_prev is not None:
                        # previous batch's finalize chain runs here: its
                        # inputs completed a full tile ago, so the PE ops
                        # inside it never stall the engine
                        fin_prev()

                if pending is not None:
                    emit_attn(*pending)
                    pending = None

                # DVE/ACT precursors right after the attention flush; the
                # PE-bearing remainder is deferred one tile into the next
                # batch so its operands are long since ready
                skp = smallp.tile([C8, 1], F32, tag="skp")
                nc.vector.scalar_tensor_tensor(
                    skp[:], bk_col[:], float(N), att_ps[0:C8, 0:1], op0=MULT, op1=ADD
                )
                sq_sb = smallp.tile([C8 + 1, C8], F32, tag="sq_sb")
                nc.scalar.copy(sq_sb[C8 : C8 + 1, :], att_ps[C8 : C8 + 1, 1 : C8 + 1])
                # v sums sit on partition 64, cols 65:129
                vrow_sb = smallp.tile([C8 + 1, C8], F32, tag="vrow_sb")
                nc.scalar.copy(
                    vrow_sb[C8 : C8 + 1, :], att_ps[C8 : C8 + 1, C8 + 1 : 2 * C8 + 1]
                )

                return lambda: finalize_batch(b, att_ps, skp, sq_sb, vrow_sb)

            def finalize_batch(b, att_ps, skp, sq_sb, vrow_sb):
                # vsum row (partition 64) -> column via rank-1 matmul at p64
                vb_ps = ps_small.tile([C8, 1], F32, tag="sp")
                nc.tensor.matmul(
                    vb_ps[:],
                    vrow_sb[C8 : C8 + 1, :],
                    ones64[C8 : C8 + 1, 0:1],
                    start=True,
                    stop=True,
                )
                vbar = smallp.tile([C8, 1], F32, tag="vbar")
                nc.vector.scalar_tensor_tensor(
                    vbar[:], vb_ps[:], 1.0 / N, bv_col[:], op0=MULT, op1=ADD
                )
                # broadcast sq (row on partition 64) to all partitions
                sq_ps = ps_small.tile([C8, C8], F32, tag="sp")
                nc.tensor.matmul(
                    sq_ps[:],
                    ones64[C8 : C8 + 1, :],
                    sq_sb[C8 : C8 + 1, :],
                    start=True,
                    stop=True,
                )
                # LT = L0T + bq_bc * skp + sq_bc * bk
                L1 = smallp.tile([C8, C8], F32, tag="L1")
                nc.vector.scalar_tensor_tensor(
                    L1[:], bq_bc[:], skp[:], att_ps[0:C8, 1 : C8 + 1],
                    op0=MULT, op1=ADD,
                )
                LT = smallp.tile([C8, C8], F32, tag="LT")
                nc.vector.scalar_tensor_tensor(
                    LT[:], sq_ps[:], bk_col[:], L1[:], op0=MULT, op1=ADD
                )
                # softmax along free dim (the o axis)
                negm = smallp.tile([C8, 1], F32, tag="negm")
                nc.vector.reduce_max(negm[:], LT[:], axis=AX, negate=True)
                E = smallp.tile([C8, C8], F32, tag="E")
                s_col = smallp.tile([C8, 1], F32, tag="s_col")
                nc.scalar.activation(
                    E[:],
                    LT[:],
                    mybir.ActivationFunctionType.Exp,
                    bias=negm[:],
                    scale=1.0,
                    accum_out=s_col[:],
                )
                # w = vbar / s ; out = E^T @ w  (as row via lhsT=w)
                rs = smallp.tile([C8, 1], F32, tag="rs")
                nc.vector.reciprocal(rs[:], s_col[:])
                wcol = smallp.tile([C8, 1], F32, tag="wcol")
                nc.vector.tensor_tensor(wcol[:], vbar[:], rs[:], op=MULT)
                out_ps = ps_small.tile([1, C8], F32, tag="sp")
                nc.tensor.matmul(out_ps[:], wcol[:], E[:], start=True, stop=True)
                out_row = smallp.tile([1, C8], F32, tag="out_row")
                nc.scalar.copy(out_row[:], out_ps[:])
                nc.gpsimd.dma_start(out_d.ap()[b : b + 1, :], out_row[:])

            if loop_n is None:
                emit_batches()
            else:
                hints = (
                    mybir.EngineType.PE,
                    mybir.EngineType.DVE,
                    mybir.EngineType.Activation,
                    mybir.EngineType.SP,
                    mybir.EngineType.Pool,
                )
                with tc.For_i(0, loop_n, 1, hint_engines=hints):
                    emit_batches()

    nc.compile()
    return nc


def _get_nc(loop_n=None, mode="full"):
    key = ("nc", loop_n, mode)
    if key not in _NC_CACHE:
        _NC_CACHE[key] = _build_nc(loop_n, mode)
    return _NC_CACHE[key]


def _make_in_maps(x, wq, bq, wk, bk, wv, bv):
    # fp16 shipping: same 10-bit mantissa as the tf32-class device compute,
    # but halves the HBM traffic for x
    xf = np.ascontiguousarray(
        np.asarray(x, dtype=np.float32).reshape(B, C, N).astype(np.float16)
    )
    # fused [wq | wv | wk] block, transposed to [C, 192] then viewed as
    # [128, 4, 192]: row c = 4p + j lands at (p, j) — matching the device's
    # (p j) n -> p j n view of x
    wall = np.concatenate(
        [
            np.asarray(wq, np.float32),
            np.asarray(wv, np.float32),
            np.asarray(wk, np.float32),
        ],
        axis=0,
    )  # [192, C]
    wqkvT = np.ascontiguousarray(
        wall.T.reshape(128, NCHUNK, 192).astype(np.float16)
    )
    shared = {
        "wqkvT": wqkvT,
        "bq": np.asarray(bq, np.float32),
        "bk": np.asarray(bk, np.float32),
        "bv": np.asarray(bv, np.float32),
    }
    return [
        {"x": xf[i * BPC : (i + 1) * BPC], **shared} for i in range(NCORES)
    ]


def kernel(x, wq, bq, wk, bk, wv, bv):
    nc = _get_nc()
    in_maps = _make_in_maps(x, wq, bq, wk, bk, wv, bv)
    res = run_bass_kernel_spmd(nc, in_maps, core_ids=list(range(NCORES)))
    out = np.concatenate([res.results[i]["out"] for i in range(NCORES)], axis=0)
    return out.astype(np.float32)


# revision 32
# speedup vs baseline: 1.1068x; 1.0189x over previous
"""Trainium2 Bass kernel for AttentionPooling.

Math (per batch element b):
  xf = x[b] reshaped [C, N] with C=512, N=4096
  q = wq@xf + bq ; k = wk@xf + bk ; v = wv@xf + bv          (each [64, N])
  logits = q @ k^T  [64, 64];  attn = softmax(logits, axis over rows o)
  out[b] = mean_n(attn @ v) = attn @ mean_n(v)              ([64])

Because attn does not depend on n, mean_n(attn @ v) = attn @ vbar with
vbar = mean_n(v) — the heavy [64, N] attn@v product collapses to a [64]
vector, so only the q/k projections and a 64x64 logits product are real
work.  Implementation, per batch element:

  - x is shipped as fp16 (10-bit mantissa, same class as tf32/f32r
    device rounding; empirically 4.5e-3 end-to-end rel err) which halves
    the HBM traffic for x — the dominant memory cost.
  - x[b] arrives in ONE 4MB dma_start into a [128, 4, 4096] SBUF tile
    via the AP view (p j) n -> p j n: partition p holds channels
    4p..4p+3, i.e. 32KB contiguous DRAM per partition -> 128 fat
    descriptors, near-peak HBM bandwidth.  The channel permutation
    c = 4p + j is folded into the host-side weight layout.
  - X-STATIONARY fused projection: each [128, 128] x chunk-subtile is
    the matmul weight; ONE matmul per (chunk, subtile) streams the fused
    [wqT | wvT | wkT] 192-column block through it, producing
    [q0T | v0T | k0T] directly in n-major layout (no transposes, half
    the PE instructions), fp32 PSUM accumulation over the 4 C-chunks.
    The weight block is pre-transposed AND channel-permuted on the host
    and shipped as one [128, 4, 192] fp16 tensor.
  - One 3D-AP vector-engine copy per projection tile moves all 4
    subtiles' [ones | qT | vT | kT | ones] attention operands to SBUF;
    the ones-augmented [65, 129] attention matmuls (lhsT=[kT|1],
    rhs=[1|qT|vT], accumulated over all 32 n-subtiles) produce L0^T AND
    sum_n k0 (column 0), sum_n q0, sum_n v0 (partition-64 row) — the
    bias-correction and pooling sums ride the same accumulation.  They
    run one projection tile behind so they never stall on the copies.
  - Bias corrections applied analytically on the 64x64 logits:
      L^T = L0^T + bq (x) (sk + N bk) + bk (x) sq
    (valid because logits(q0+bq, k0+bk) is bilinear and attn/softmax only
    needs the full L^T).  sq is broadcast and the v-sum row transposed
    off PSUM partition 64 with rank-1 matmuls whose operands both live
    on partition 64.
  - Softmax along the free dim of L^T (scalar-engine exp with accumulated
    denominator), folded:  out = E^T @ (vbar / s) as one [64]x[64,64]
    matmul producing the output row directly.

Data-parallel over batch across the 8 NeuronCores (4 batch elements per
core); no collectives needed.
"""

import sys

import numpy as np

for _p in ("/opt/trn_rl_repo", "/root/.axon_site/_ro/trn_rl_repo"):
    if _p not in sys.path:
        sys.path.insert(0, _p)

import concourse.bacc as bacc
import concourse.mybir as mybir
import concourse.tile as tile
from concourse.bass_utils import run_bass_kernel_spmd

B, C, H, W = 32, 512, 64, 64
N = H * W            # 4096
C8 = 64              # C // 8
NCORES = 8
BPC = B // NCORES    # batch elements per core
NCHUNK = C // 128    # C chunks of 128
TW = 512             # projection tile width (PSUM bank = 512 f32)
NT = N // TW         # 8 projection tiles
NSUB = TW // 128     # transpose subtiles per projection tile

F32 = mybir.dt.float32
F16 = mybir.dt.float16
AX = mybir.AxisListType.X
MULT = mybir.AluOpType.mult
ADD = mybir.AluOpType.add

_NC_CACHE = {}


def _build_nc(loop_n=None, mode="full"):
    """Build the bass program.  loop_n wraps the per-batch section in a
    device-side For_i loop (used only for timing: the NEFF then executes the
    whole workload loop_n times back-to-back, making device time measurable
    over the host dispatch overhead).  mode: "full" | "dma" (x loads only)
    | "compute" (batch-0 x loaded once outside the loop, engines only)."""
    nc = bacc.Bacc("TRN2", target_bir_lowering=False, debug=False)

    x_d = nc.dram_tensor("x", [BPC, C, N], F16, kind="ExternalInput")
    wqkvT_d = nc.dram_tensor("wqkvT", [128, NCHUNK, 192], F16, kind="ExternalInput")
    bq_d = nc.dram_tensor("bq", [C8], F32, kind="ExternalInput")
    bk_d = nc.dram_tensor("bk", [C8], F32, kind="ExternalInput")
    bv_d = nc.dram_tensor("bv", [C8], F32, kind="ExternalInput")
    out_d = nc.dram_tensor("out", [BPC, C8], F32, kind="ExternalOutput")

    with tile.TileContext(nc, trace_sim=False) as tc:
        with (
            tc.tile_pool(name="const", bufs=1) as constp,
            tc.tile_pool(name="xpool", bufs=2) as xpool,
            tc.tile_pool(name="attpool", bufs=3) as attpool,
            tc.tile_pool(name="smallp", bufs=2) as smallp,
            tc.tile_pool(name="ps_qk", bufs=2, space="PSUM") as ps_qk,
            tc.tile_pool(name="ps_att", bufs=2, space="PSUM") as ps_att,
            tc.tile_pool(name="ps_small", bufs=1, space="PSUM") as ps_small,
        ):
            # ---------------- one-time prep ----------------
            ones_row = constp.tile([1, C8], F32)
            nc.vector.memset(ones_row[:], 1.0)
            ones2_f32 = constp.tile([128, 2], F32)
            nc.vector.memset(ones2_f32[:], 1.0)
            ones2_16 = constp.tile([128, 2], F16)
            nc.scalar.copy(ones2_16[:], ones2_f32[:])
            # ones at partition 64 (to broadcast the sq row the attention
            # matmul leaves on PSUM partition 64)
            ones64 = constp.tile([C8 + 1, C8], F32)
            nc.vector.memset(ones64[C8 : C8 + 1, :], 1.0)

            # pre-transposed, channel-permuted fused weights [p, chunk, 192]
            wsb = constp.tile([128, NCHUNK, 192], F16)
            nc.sync.dma_start(wsb[:], wqkvT_d.ap()[:, :, :])

            bq_row = constp.tile([1, C8], F32)
            nc.sync.dma_start(bq_row[:], bq_d.ap().unsqueeze(0))
            bk_row = constp.tile([1, C8], F32)
            nc.sync.dma_start(bk_row[:], bk_d.ap().unsqueeze(0))
            bv_row = constp.tile([1, C8], F32)
            nc.sync.dma_start(bv_row[:], bv_d.ap().unsqueeze(0))

            # bias-derived constants
            p_bc = ps_small.tile([C8, C8], F32, tag="sp")
            nc.tensor.matmul(p_bc[:], ones_row[:], bq_row[:], start=True, stop=True)
            bq_bc = constp.tile([C8, C8], F32)  # every row = bq
            nc.scalar.copy(bq_bc[:], p_bc[:])

            p_bk = ps_small.tile([C8, 1], F32, tag="sp")
            nc.tensor.matmul(
                p_bk[:], bk_row[:], ones_row[:, 0:1], start=True, stop=True
            )
            bk_col = constp.tile([C8, 1], F32)
            nc.scalar.copy(bk_col[:], p_bk[:])

            p_bv = ps_small.tile([C8, 1], F32, tag="sp")
            nc.tensor.matmul(
                p_bv[:], bv_row[:], ones_row[:, 0:1], start=True, stop=True
            )
            bv_col = constp.tile([C8, 1], F32)
            nc.scalar.copy(bv_col[:], p_bv[:])

            # ---------------- per batch element ----------------
            XSPLIT = 4

            def dma_batch_into(b, pool, tagp, first_small=False):
                # [128, 4, w] fp16 tiles; partition p holds channels
                # 4p..4p+3.  The For_i timing loop drains the whole pipeline
                # at each iteration boundary, so x moves as ~1MB pieces in
                # strict consumption order: the exposed head DMA is one
                # piece, and no big transfer head-of-line-blocks a piece
                # needed sooner.  Batch 0's first MB goes as two 512KB
                # pieces to shorten the post-barrier head further.  Pieces
                # alternate between the SP and ACT HWDGE rings.
                widths = [N // XSPLIT] * XSPLIT
                if first_small:
                    widths = [N // 8, N // 8] + widths[1:]
                ts = []
                col = 0
                for i, w in enumerate(widths):
                    t = pool.tile(
                        [128, NCHUNK, w], F16, tag=f"{tagp}s{i}w{w}"
                    )
                    eng = nc.sync if (b * XSPLIT + i) % 2 == 0 else nc.scalar
                    eng.dma_start(
                        t[:],
                        x_d.ap()[b].rearrange("(p j) n -> p j n", j=NCHUNK)[
                            :, :, col : col + w
                        ],
                    )
                    ts.append((col, w, t))
                    col += w
                return ts

            xc_static = None
            if mode == "compute":
                xc_static = dma_batch_into(0, constp, "xs")

            def dma_batch(b):
                return dma_batch_into(b, xpool, "x", first_small=(b == 0))

            def emit_batches():
                if mode == "dma":
                    for b in range(BPC):
                        dma_batch(b)
                    return
                if mode == "compute":
                    fin = None
                    for b in range(BPC):
                        fin = emit_batch(b, xc_static, fin)
                    fin()
                    return
                xc_next = dma_batch(0)
                fin = None
                for b in range(BPC):
                    xc_cur = xc_next
                    if b + 1 < BPC:
                        xc_next = dma_batch(b + 1)
                    fin = emit_batch(b, xc_cur, fin)
                fin()

            def emit_attn(att_ps, ti, a_list):
                for s in range(NSUB):
                    first = ti == 0 and s == 0
                    last = ti == NT - 1 and s == NSUB - 1
                    # lhsT=[kT|1], rhs=[1|qT] -> out[65,65]:
                    #   [0:64, 0] = sk, [0:64, 1:65] = L0T,
                    #   [64, 1:65] = sq, [64, 0] = N
                    nc.tensor.matmul(
                        att_ps[:],
                        a_list[:, s, 129:194],
                        a_list[:, s, 0:129],
                        start=first,
                        stop=last,
                    )

            def emit_batch(b, xt, fin_prev):
                pending = None

                # [65, 129]: [0:64,0]=sk, [0:64,1:65]=L0T, [64,1:65]=sq,
                # [64,65:129]=sum_n v0 (accumulated over all subtiles)
                att_ps = ps_att.tile([C8 + 1, 2 * C8 + 1], F32)

                xts = xt
                for ti in range(NT):
                    base = ti * TW
                    # x-stationary fused projection: one matmul per
                    # (chunk, subtile) streams [wqT | wvT | wkT] through the
                    # stationary x chunk -> [qT | vT | kT] in n-major layout
                    qk_ps = ps_qk.tile([128, NSUB, 256], F32, tag="qk_ps")
                    for s in range(NSUB):
                        n0 = base + s * 128
                        col, w, xti = next(
                            (c, w_, t_)
                            for (c, w_, t_) in xts
                            if c <= n0 < c + w_
                        )
                        nsl = slice(n0 - col, n0 - col + 128)
                        for j in range(NCHUNK):
                            nc.tensor.matmul(
                                qk_ps[:, s, 0:192],
                                xti[:, j, nsl],
                                wsb[:, j, :],
                                start=(j == 0),
                                stop=(j == NCHUNK - 1),
                            )

                    # one [128, 4x194] tile holds all 4 subtiles' attention
                    # operands [ones | qT | vT | kT | ones]; single 3D copies
                    a_sb = attpool.tile([128, NSUB, 194], F16, tag="a_sb")
                    nc.vector.tensor_copy(a_sb[:, :, 1:193], qk_ps[:, :, 0:192])
                    nc.vector.tensor_copy(
                        a_sb[:, :, 0:194:193],
                        ones2_16[:].unsqueeze(1).broadcast_to([128, NSUB, 2]),
                    )
                    a_list = a_sb
                    # attention matmuls run one projection tile behind, so
                    # their a_sb inputs were copied a whole tile ago (no PE
                    # stall on the DVE copy)
                    if pending is not None:
                        emit_attn(*pending)
                    pending = (att_ps, ti, a_list)
                    if ti == 1 and fin_prev is not None:
                        # previous batch's finalize chain runs here: its
                        # inputs completed a full tile ago, so the PE ops
                        # inside it never stall the engine
                        fin_prev()

                if pending is not None:
                    emit_attn(*pending)
                    pending = None

                # DVE/ACT precursors right after the attention flush; the
                # PE-bearing remainder is deferred one tile into the next
                # batch so its operands are long since ready
                skp = smallp.tile([C8, 1], F32, tag="skp")
                nc.vector.scalar_tensor_tensor(
                    skp[:], bk_col[:], float(N), att_ps[0:C8, 0:1], op0=MULT, op1=ADD
                )
                sq_sb = smallp.tile([C8 + 1, C8], F32, tag="sq_sb")
                nc.scalar.copy(sq_sb[C8 : C8 + 1, :], att_ps[C8 : C8 + 1, 1 : C8 + 1])
                # v sums sit on partition 64, cols 65:129
                vrow_sb = smallp.tile([C8 + 1, C8], F32, tag="vrow_sb")
                nc.scalar.copy(
                    vrow_sb[C8 : C8 + 1, :], att_ps[C8 : C8 + 1, C8 + 1 : 2 * C8 + 1]
                )

                return lambda: finalize_batch(b, att_ps, skp, sq_sb, vrow_sb)

            def finalize_batch(b, att_ps, skp, sq_sb, vrow_sb):
                # vsum row (partition 64) -> column via rank-1 matmul at p64
                vb_ps = ps_small.tile([C8, 1], F32, tag="sp")
                nc.tensor.matmul(
                    vb_ps[:],
                    vrow_sb[C8 : C8 + 1, :],
                    ones64[C8 : C8 + 1, 0:1],
                    start=True,
                    stop=True,
                )
                vbar = smallp.tile([C8, 1], F32, tag="vbar")
                nc.vector.scalar_tensor_tensor(
                    vbar[:], vb_ps[:], 1.0 / N, bv_col[:], op0=MULT, op1=ADD
                )
                # broadcast sq (row on partition 64) to all partitions
                sq_ps = ps_small.tile([C8, C8], F32, tag="sp")
                nc.tensor.matmul(
                    sq_ps[:],
                    ones64[C8 : C8 + 1, :],
                    sq_sb[C8 : C8 + 1, :],
                    start=True,
                    stop=True,
                )
                # LT = L0T + bq_bc * skp + sq_bc * bk
                L1 = smallp.tile([C8, C8], F32, tag="L1")
                nc.vector.scalar_tensor_tensor(
                    L1[:], bq_bc[:], skp[:], att_ps[0:C8, 1 : C8 + 1],
                    op0=MULT, op1=ADD,
                )
                LT = smallp.tile([C8, C8], F32, tag="LT")
                nc.vector.scalar_tensor_tensor(
                    LT[:], sq_ps[:], bk_col[:], L1[:], op0=MULT, op1=ADD
                )
                # softmax along free dim (the o axis)
                negm = smallp.tile([C8, 1], F32, tag="negm")
                nc.vector.reduce_max(negm[:], LT[:], axis=AX, negate=True)
                E = smallp.tile([C8, C8], F32, tag="E")
                s_col = smallp.tile([C8, 1], F32, tag="s_col")
                nc.scalar.activation(
                    E[:],
                    LT[:],
                    mybir.ActivationFunctionType.Exp,
                    bias=negm[:],
                    scale=1.0,
                    accum_out=s_col[:],
                )
                # w = vbar / s ; out = E^T @ w  (as row via lhsT=w)
                rs = smallp.tile([C8, 1], F32, tag="rs")
                nc.vector.reciprocal(rs[:], s_col[:])
                wcol = smallp.tile([C8, 1], F32, tag="wcol")
                nc.vector.tensor_tensor(wcol[:], vbar[:], rs[:], op=MULT)
                out_ps = ps_small.tile([1, C8], F32, tag="sp")
                nc.tensor.matmul(out_ps[:], wcol[:], E[:], start=True, stop=True)
                out_row = smallp.tile([1, C8], F32, tag="out_row")
                nc.scalar.copy(out_row[:], out_ps[:])
                eng = nc.sync if b % 2 == 0 else nc.scalar
                eng.dma_start(out_d.ap()[b : b + 1, :], out_row[:])

            if loop_n is None:
                emit_batches()
            else:
                hints = (
                    mybir.EngineType.PE,
                    mybir.EngineType.DVE,
                    mybir.EngineType.Activation,
                    mybir.EngineType.SP,
                    mybir.EngineType.Pool,
                )
                with tc.For_i(0, loop_n, 1, hint_engines=hints):
                    emit_batches()

    nc.compile()
    return nc


def _get_nc(loop_n=None, mode="full"):
    key = ("nc", loop_n, mode)
    if key not in _NC_CACHE:
        _NC_CACHE[key] = _build_nc(loop_n, mode)
    return _NC_CACHE[key]


def _make_in_maps(x, wq, bq, wk, bk, wv, bv):
    # fp16 shipping: same 10-bit mantissa as the tf32-class device compute,
    # but halves the HBM traffic for x
    xf = np.ascontiguousarray(
        np.asarray(x, dtype=np.float32).reshape(B, C, N).astype(np.float16)
    )
    # fused [wq | wv | wk] block, transposed to [C, 192] then viewed as
    # [128, 4, 192]: row c = 4p + j lands at (p, j) — matching the device's
    # (p j) n -> p j n view of x
    wall = np.concatenate(
        [
            np.asarray(wq, np.float32),
            np.asarray(wv, np.float32),
            np.asarray(wk, np.float32),
        ],
        axis=0,
    )  # [192, C]
    wqkvT = np.ascontiguousarray(
        wall.T.reshape(128, NCHUNK, 192).astype(np.float16)
    )
    shared = {
        "wqkvT": wqkvT,
        "bq": np.asarray(bq, np.float32),
        "bk": np.asarray(bk, np.float32),
        "bv": np.asarray(bv, np.float32),
    }
    return [
        {"x": xf[i * BPC : (i + 1) * BPC], **shared} for i in range(NCORES)
    ]


def kernel(x, wq, bq, wk, bk, wv, bv):
    nc = _get_nc()
    in_maps = _make_in_maps(x, wq, bq, wk, bk, wv, bv)
    res = run_bass_kernel_spmd(nc, in_maps, core_ids=list(range(NCORES)))
    out = np.concatenate([res.results[i]["out"] for i in range(NCORES)], axis=0)
    return out.astype(np.float32)
